# revision 19
# baseline (speedup 1.0000x reference)
"""Two-layer GCN (BotGCN) on 8 Trainium2 NeuronCores.

Distribution: nodes partitioned contiguously across the 8 cores (12500
each). Each core owns the edges whose destination lands in its block.
Layer math is refactored so all per-edge work is a gather of pre-scaled
rows + a segment-sum:

    out[v] = dinv[v] * (sum_{e: dst=v, real} (dinv[src] * h[src])
                        + dinv[v] * h[v]) + bias

Self-loops are the elementwise term dinv[v]^2 * h[v], added from an
SBUF-resident copy of the local pre-scaled rows.

Structure:
  - Gather table is PIECE-major: local nodes split into 4 pieces
    (3200/3200/3200/2900 + 2 zero rows per core); piece p of the table
    is [8 x PSZE[p] x 256B], produced by its own AllGather so the
    collectives stream piece-by-piece and overlap with compute.
  - Destinations processed in 4 passes of 25/25/25/23 blocks aligned
    with the pieces; layer-1 post for pass p feeds AllGather piece p of
    layer 2 while later passes still compute. Explicit dependency edges
    pin each AG trigger early in the Pool stream (the collective must
    live on the Pool engine) so its ~40us mesh hides under compute.
  - Slots are SEGMENT-packed: per (block, piece) segment rounded to 16
    slots (max over cores), segments concatenated per (pass, piece)
    section and the section padded to 128. 128-slot groups may span two
    blocks; each (group, block) pair gets its own matmul whose one-hot
    is generated on-chip: is_equal(code[slot], bip*128 + c) with
    code = block-in-pass*128 + dst%128 (f32 exact), against an f32
    iota-base table. ~11% padding vs ~21% for per-(block,piece)
    rounding to 128.
  - One DVE is_equal per matmul, 4 dma_gathers per chunk (one per SWDGE
    queue), PE matmuls accumulate into per-pass PSUM banks.
"""

import numpy as np

N = 100000
NCORES = 8
NPC = N // NCORES            # 12500 nodes per core
BLK = 128
NBLK = (NPC + BLK - 1) // BLK          # 98 destination blocks
LAST_BLK = NPC - (NBLK - 1) * BLK      # 84 nodes in the last block
F_IN, F_HID, F_OUT = 128, 64, 2
TBLW = 128                              # bf16 table row width (256B)
NP_ = 4                                 # pieces (gather windows)
NPASS = 6                               # destination passes
PASS_BLOCKS = [list(range(0, 17)), list(range(17, 34)),
               list(range(34, 50)), list(range(50, 66)),
               list(range(66, 82)), list(range(82, NBLK))]
# AG2 piece pp (blocks [25pp,25pp+25)) completes after these passes
AG2_AFTER_PASS = {0: 0, 2: 1, 4: 2, 5: 3}  # pass -> piece
PBLK = [16, 26, 28, 28]                 # blocks per piece: sized so each
                                        # window's gather work covers the
                                        # next piece's AllGather mesh
PBND = [0, 16, 42, 70, 98]              # piece block boundaries
PSTART = [0, 2048, 5376, 8960]          # local node offset of each piece
PSZ = [2048, 3328, 3584, 3540]          # local nodes per piece
PSZE = [q + 2 for q in PSZ]             # + 2 zero rows per core shard
CHUNK_GROUPS = 24                       # max 128-slot groups per chunk

_CACHE = {}

# Results of the most recent run (for the local test harness's profiling).
LAST_RESULTS = None


def _preprocess(edge_index):
    """Host-side integer bucketing of the edge list (self-loops excluded).

    Per (block, piece) segment sized to the max over cores, rounded to
    16 slots; segments concatenated per (pass, piece) section, section
    padded to a 128 multiple. Returns the section descriptors, slot
    count S, and per-core staged gather-index / code arrays.
    """
    src = np.asarray(edge_index[0]).astype(np.int64)
    dst = np.asarray(edge_index[1]).astype(np.int64)

    # degree includes the self-loop (reference semantics)
    deg = (np.bincount(dst, minlength=N) + 1).astype(np.float32)

    core = dst // NPC
    dloc = dst % NPC
    blk = dloc // BLK
    scor = src // NPC
    sloc = src % NPC
    piece = np.searchsorted(np.array(PSTART[1:] + [NPC]), sloc,
                            side="right")
    piece = np.minimum(piece, NP_ - 1)

    cnt = np.zeros((NCORES, NBLK, NP_), np.int64)
    np.add.at(cnt, (core, blk, piece), 1)
    seg = (-(-cnt.max(axis=0) // 16) * 16).astype(np.int64)   # [98, 4]

    seg_id_of = np.zeros((NBLK, NP_), np.int64)
    seg_sizes = []
    seg_win = []
    sections = []
    sid = 0
    slot_base = 0
    for p in range(NPASS):
        for w in range(NP_):
            sec_start = slot_base
            bounds = []
            off = 0
            for b in PASS_BLOCKS[p]:
                seg_id_of[b, w] = sid
                seg_sizes.append(int(seg[b, w]))
                seg_win.append(w)
                bounds.append((b, off, off + int(seg[b, w])))
                off += int(seg[b, w])
                sid += 1
            tot128 = -(-off // 128) * 128
            if tot128 > off:
                seg_sizes.append(tot128 - off)   # section pad pseudo-segment
                seg_win.append(w)
                sid += 1
            slot_base += tot128
            ngroups = tot128 // 128
            mms = []
            for j in range(ngroups):
                lo, hi = j * 128, (j + 1) * 128
                for (b, s0, s1) in bounds:
                    if s0 < hi and s1 > lo:
                        mms.append((j, b))
            byj = {}
            for (j, b) in mms:
                byj.setdefault(j, []).append(b)
            mms_first, mms_cross = [], []
            for j in sorted(byj):
                bs = byj[j]
                assert len(bs) <= 2, bs
                mms_first.append((j, bs[0]))
                if len(bs) > 1:
                    mms_cross.append((j, bs[1]))
            sections.append({"pass": p, "w": w, "ngroups": ngroups,
                             "mms_first": mms_first,
                             "mms_cross": mms_cross,
                             "start_slot": sec_start})
    # processing order: pair passes (0,1),(2,3),(4,5); within a pair,
    # sweep windows and interleave the two passes so each window gives
    # ~2 sections of work to hide one CC AllGather mesh (~42us).
    by_pw = {(sec["pass"], sec["w"]): sec for sec in sections}
    sections = [by_pw[(pp + d, w)]
                for pp in range(0, NPASS, 2)
                for w in range(NP_)
                for d in (0, 1)]
    S = slot_base
    nseg = sid
    seg_sizes = np.array(seg_sizes, np.int64)
    seg_win = np.array(seg_win, np.int64)
    seg_offs = np.zeros(nseg + 1, np.int64)
    np.cumsum(seg_sizes, out=seg_offs[1:])
    win_of_slot = np.repeat(seg_win, seg_sizes)
    zrow = np.array([PSZ[w] for w in range(NP_)], np.int64)

    # per-group base code (first overlapping block-in-pass * 128)
    gbase = np.zeros(S // BLK, np.float32)

    bip = np.zeros(NBLK, np.int64)
    for p in range(NPASS):
        for i, b in enumerate(PASS_BLOCKS[p]):
            bip[b] = i
    for sec in sections:
        g0 = sec["start_slot"] // BLK
        for (j, b) in sec["mms_first"]:
            gbase[g0 + j] = bip[b] * BLK
    base_of_slot = np.repeat(gbase, BLK)

    per_core = []
    for c in range(NCORES):
        m = core == c
        key = seg_id_of[blk[m], piece[m]]
        order = np.argsort(key, kind="stable")
        ks = key[order]
        bstart = np.searchsorted(ks, np.arange(nseg))
        rank = np.arange(len(ks)) - bstart[ks]
        slot = seg_offs[ks] + rank

        so_cor = scor[m][order]
        so_loc = sloc[m][order]
        so_p = piece[m][order]
        row = so_cor * np.array(PSZE)[so_p] + (so_loc - np.array(PSTART)[so_p])

        gidx = zrow[win_of_slot].astype(np.int16)   # pad -> window zero row
        gidx[slot] = row.astype(np.int16)
        code = np.full(S, -1.0, np.float32)
        code[slot] = (bip[blk[m][order]] * BLK
                      + dloc[m][order] % BLK).astype(np.float32)
        # shifted codes: cs1 selects the group's first block (values in
        # [0,128) there), cs2 the second; everything else falls outside
        # [0,128) so the chunk-wide iota is_equal gives all-zero rows.
        cs1 = (code - base_of_slot).astype(np.int16)
        cs2 = (cs1 - BLK).astype(np.int16)

        gidx16 = gidx.reshape(S // 16, 16).T      # [16, S/16]
        gidx_rep = np.tile(gidx16, (8, 1)).copy() # replicated for Q7 cores
        # per slot, [128, S/128]: partition = slot % 128, col = group
        cs1G = np.ascontiguousarray(cs1.reshape(S // BLK, BLK).T)
        cs2G = np.ascontiguousarray(cs2.reshape(S // BLK, BLK).T)
        assert cs1G.dtype == np.int16 and cs2G.dtype == np.int16

        degc = np.ones(NBLK * BLK, np.float32)
        degc[:NPC] = deg[c * NPC:(c + 1) * NPC]
        degT = degc.reshape(NBLK, BLK).T.copy()   # [128, NBLK]

        per_core.append({"gidx": gidx_rep, "cs1G": cs1G, "cs2G": cs2G,
                         "degT": degT})

    return sections, S, per_core


def _chunks_of(n):
    """Split n groups into near-equal chunks of at most CHUNK_GROUPS."""
    k = -(-n // CHUNK_GROUPS)
    base, rem = divmod(n, k)
    out = []
    s = 0
    for i in range(k):
        sz = base + (1 if i < rem else 0)
        out.append((s, sz))
        s += sz
    return out


def _build(sections, S, b1_nonzero, b2_nonzero):
    import concourse.bacc as bacc
    import concourse.mybir as mybir
    import concourse.tile as tile
    from concourse.masks import make_identity
    from bass_rust import add_dep_helper

    f32 = mybir.dt.float32
    bf16 = mybir.dt.bfloat16
    AT = mybir.AluOpType

    bip = {}
    pos = {}
    for p in range(NPASS):
        for i, b in enumerate(PASS_BLOCKS[p]):
            bip[b] = i
            pos[b] = divmod(i, 8)

    # first/last matmul (global emission index) per (pass, bank);
    # emission order: per chunk, first-block matmuls then crossings.
    first, last = {}, {}
    mmidx = 0
    for sec in sections:
        p = sec["pass"]
        for (cs, cn) in _chunks_of(sec["ngroups"]):
            for mlist in (sec["mms_first"], sec["mms_cross"]):
                for (j, b) in mlist:
                    if cs <= j < cs + cn:
                        bank, _ = pos[b]
                        last[(p, bank)] = mmidx
                        first.setdefault((p, bank), mmidx)
                        mmidx += 1

    nc = bacc.Bacc("TRN2", target_bir_lowering=False, debug=False,
                   enable_asserts=False, num_devices=NCORES,
                   num_swdge_queues=4)
    xT = nc.dram_tensor("xT", [F_IN, NPC], bf16, kind="ExternalInput")
    W1 = nc.dram_tensor("W1", [F_IN, F_HID], bf16, kind="ExternalInput")
    W2 = nc.dram_tensor("W2", [F_HID, F_OUT], f32, kind="ExternalInput")
    b1r = nc.dram_tensor("b1r", [BLK, F_HID], f32, kind="ExternalInput")
    b2r = nc.dram_tensor("b2r", [BLK, F_OUT], f32, kind="ExternalInput")
    degT = nc.dram_tensor("degT", [BLK, NBLK], f32, kind="ExternalInput")
    gidx = nc.dram_tensor("gidx", [BLK, S // 16], mybir.dt.int16,
                          kind="ExternalInput")
    cs1G = nc.dram_tensor("cs1G", [BLK, S // BLK], mybir.dt.int16,
                          kind="ExternalInput")
    cs2G = nc.dram_tensor("cs2G", [BLK, S // BLK], mybir.dt.int16,
                          kind="ExternalInput")
    iotab = nc.dram_tensor("iotab", [BLK, BLK], mybir.dt.int16,
                           kind="ExternalInput")
    y = nc.dram_tensor("y", [NPC, F_OUT], f32, kind="ExternalOutput")

    with tile.TileContext(nc) as tc:
        with tc.tile_pool(name="const", bufs=1) as const, \
             tc.tile_pool(name="xt", bufs=3) as xpool, \
             tc.tile_pool(name="hs", bufs=3) as hpool, \
             tc.tile_pool(name="msgs", bufs=12) as mpool, \
             tc.tile_pool(name="oh", bufs=4) as ohpool, \
             tc.tile_pool(name="post", bufs=3) as ppool, \
             tc.tile_pool(name="psb", bufs=2, space="PSUM") as psb, \
             tc.tile_pool(name="pst", bufs=2, space="PSUM") as pst, \
             tc.tile_pool(name="dram", bufs=1, space="DRAM") as dram:

            ag_in = [[dram.tile([PSZE[p], TBLW], bf16,
                                name=f"agin{L}_{p}", tag=f"agin{L}_{p}")
                      for p in range(NP_)] for L in range(2)]
            ag_out = [[dram.tile([8 * PSZE[p], TBLW], bf16,
                                 addr_space="Shared",
                                 name=f"agout{L}_{p}", tag=f"agout{L}_{p}")
                       for p in range(NP_)] for L in range(2)]

            # ---- constants ----
            ident = const.tile([BLK, BLK], f32)
            make_identity(nc, ident[:])
            W1t = const.tile([F_IN, F_HID], bf16)
            nc.sync.dma_start(W1t[:], W1[:])
            W2t = const.tile([F_HID, F_OUT], f32)
            nc.sync.dma_start(W2t[:], W2[:])
            if b1_nonzero:
                b1t = const.tile([BLK, F_HID], f32)
                nc.sync.dma_start(b1t[:], b1r[:])
            if b2_nonzero:
                b2t = const.tile([BLK, F_OUT], f32)
                nc.sync.dma_start(b2t[:], b2r[:])
            degt = const.tile([BLK, NBLK], f32)
            nc.sync.dma_start(degt[:], degT[:])
            rcp = const.tile([BLK, NBLK], f32)
            nc.vector.reciprocal(rcp[:], degt[:])
            dinv = const.tile([BLK, NBLK], f32)
            nc.scalar.sqrt(dinv[:], rcp[:])
            dinv2 = const.tile([BLK, NBLK], f32)
            nc.vector.tensor_mul(dinv2[:], dinv[:], dinv[:])
            idx_sb = const.tile([BLK, S // 16], mybir.dt.int16)
            nc.sync.dma_start(idx_sb[:], gidx[:])
            cs1_sb = const.tile([BLK, S // BLK], mybir.dt.int16)
            nc.sync.dma_start(cs1_sb[:], cs1G[:])
            cs2_sb = const.tile([BLK, S // BLK], mybir.dt.int16)
            nc.sync.dma_start(cs2_sb[:], cs2G[:])
            iota_sb = const.tile([BLK, BLK], mybir.dt.int16)
            nc.sync.dma_start(iota_sb[:], iotab[:])
            zt = const.tile([2, TBLW], bf16)
            nc.gpsimd.memset(zt[:], 0.0)
            for L in range(2):
                for p in range(NP_):
                    nc.sync.dma_start(
                        ag_in[L][p][PSZ[p]:PSZ[p] + 2, :], zt[:])

            # SBUF-resident fp32 copies of the local pre-scaled rows for
            # the elementwise self-loop term (dinv^2 * h == dinv * hs).
            hs1_all = const.tile([BLK, NBLK * F_HID], f32)
            hs2_all = const.tile([BLK, NBLK * F_HID], f32)
            nc.gpsimd.memset(hs1_all[:], 0.0)
            nc.gpsimd.memset(hs2_all[:], 0.0)

            def piece_of_block(b):
                for pp in range(NP_):
                    if b < PBND[pp + 1]:
                        return pp
                return NP_ - 1

            ag1_insts = {}
            ag2_insts = {}

            # ---- phase 1: h_scaled = dinv * (x @ W1), locally owned ----
            for p in range(NP_):
                for t in range(PBND[p], PBND[p + 1]):
                    nt = BLK if t < NBLK - 1 else LAST_BLK
                    xt = xpool.tile([F_IN, BLK], bf16, tag="xt")
                    nc.sync.dma_start(xt[:, :nt], xT[:, t * BLK:t * BLK + nt])
                    hp = pst.tile([BLK, 512], f32, space="PSUM", tag="tmp",
                                  name="hp")
                    nc.tensor.matmul(out=hp[:nt, :F_HID], lhsT=xt[:, :nt],
                                     rhs=W1t[:], start=True, stop=True)
                    nc.vector.tensor_scalar(
                        out=hs1_all[:nt, t * F_HID:(t + 1) * F_HID],
                        in0=hp[:nt, :F_HID],
                        scalar1=dinv[:nt, t:t + 1], scalar2=None,
                        op0=AT.mult)
                    hsb = hpool.tile([BLK, TBLW], bf16, tag="hs")
                    nc.scalar.activation(
                        hsb[:nt, :F_HID],
                        hs1_all[:nt, t * F_HID:(t + 1) * F_HID],
                        func=mybir.ActivationFunctionType.Copy)
                    r0 = t * BLK - PSTART[p]
                    nc.sync.dma_start(ag_in[0][p][r0:r0 + nt, :], hsb[:nt, :])
                cc1 = nc.gpsimd.collective_compute(
                    "AllGather", AT.bypass,
                    replica_groups=[list(range(NCORES))],
                    ins=[ag_in[0][p].opt()],
                    outs=[ag_out[0][p].opt()],
                )
                ag1_insts[p] = cc1

            anchor = {}       # (L, pass, w, chunk) -> first gather inst

            def run_layer(L, post_fn, after_pass=None):
                tables = ag_out[L]
                mmcount = [0]
                banks = {}

                for sec in sections:
                    p, w = sec["pass"], sec["w"]
                    blocks = PASS_BLOCKS[p]
                    if w == 0:
                        banks.clear()
                        for b in blocks:
                            bank, _ = pos[b]
                            if bank not in banks:
                                banks[bank] = psb.tile(
                                    [BLK, 512], f32, space="PSUM",
                                    name=f"bank{bank}", tag=f"bank{bank}")
                    g0_global = sec["start_slot"] // BLK
                    chunks = _chunks_of(sec["ngroups"])
                    for ci, (cs, cn) in enumerate(chunks):
                        mt = mpool.tile([BLK, CHUNK_GROUPS, TBLW], bf16,
                                        tag="msgs")
                        nsub = min(4, cn)
                        base, rem = divmod(cn, nsub)
                        j0 = 0
                        for si in range(nsub):
                            sg = base + (1 if si < rem else 0)
                            if sg == 0:
                                continue
                            sn = sg * BLK
                            soff = g0_global + cs + j0
                            gi = nc.gpsimd.dma_gather(
                                out_ap=mt[:, j0:j0 + sg, :],
                                in_ap=tables[w][:, :],
                                idxs_ap=idx_sb[:, soff * 8:
                                               soff * 8 + sn // 16],
                                num_idxs=sn, num_idxs_reg=sn,
                                elem_size=TBLW,
                                single_packet=False,
                                queue_num=si,
                            )
                            if si == 0:
                                anchor[(L, p, w, ci)] = gi
                            j0 += sg
                        firsts = [(j, b) for (j, b) in sec["mms_first"]
                                  if cs <= j < cs + cn]
                        crosses = [(j, b) for (j, b) in sec["mms_cross"]
                                   if cs <= j < cs + cn]
                        ib = iota_sb[:, :].unsqueeze(1)
                        oh1 = ohpool.tile([BLK, CHUNK_GROUPS, BLK], bf16,
                                          tag="oh1")
                        nc.vector.tensor_tensor(
                            out=oh1[:, :cn, :],
                            in0=ib.to_broadcast([BLK, cn, BLK]),
                            in1=cs1_sb[:, g0_global + cs:
                                       g0_global + cs + cn]
                                .to_broadcast([BLK, cn, BLK]),
                            op=AT.is_equal)
                        if crosses:
                            oh2 = ohpool.tile([BLK, CHUNK_GROUPS, BLK],
                                              bf16, tag="oh2")
                            nc.vector.tensor_tensor(
                                out=oh2[:, :cn, :],
                                in0=ib.to_broadcast([BLK, cn, BLK]),
                                in1=cs2_sb[:, g0_global + cs:
                                           g0_global + cs + cn]
                                    .to_broadcast([BLK, cn, BLK]),
                                op=AT.is_equal)
                        pairs = [(oh1, firsts)]
                        if crosses:
                            pairs.append((oh2, crosses))
                        for oh, mlist in pairs:
                            for (j, b) in mlist:
                                bank, off = pos[b]
                                gm = mmcount[0]
                                mmcount[0] += 1
                                nc.tensor.matmul(
                                    out=banks[bank][:, off * F_HID:
                                                    (off + 1) * F_HID],
                                    lhsT=oh[:, j - cs, :],
                                    rhs=mt[:, j - cs, :F_HID],
                                    start=(gm == first[(p, bank)]),
                                    stop=(gm == last[(p, bank)]),
                                    skip_group_check=True)
                    if w == NP_ - 1:
                        for bank, bt in banks.items():
                            bank_blocks = [b for b in blocks
                                           if pos[b][0] == bank]
                            post_fn(bank, bt, bank_blocks)
                        del pass_banks[p]
                        if after_pass is not None and p in AG2_AFTER_PASS:
                            after_pass(AG2_AFTER_PASS[p])

            # ---- layer 1 post:
            # X = bank + dinv*hs1 (self-loop); h1s = dinv*relu(dinv*X + b1)
            # b1 == 0 fast path: dinv*relu(dinv*X) == dinv^2*relu(X).
            def post1(bank, bt, bank_blocks):
                for i, b in enumerate(bank_blocks):
                    nb = BLK if b < NBLK - 1 else LAST_BLK
                    sl = bt[:, i * F_HID:(i + 1) * F_HID]
                    hb = hs1_all[:, b * F_HID:(b + 1) * F_HID]
                    x = ppool.tile([BLK, F_HID], f32, tag="post1x", name="x")
                    nc.vector.tensor_add(out=x[:], in0=hb, in1=sl)
                    sl2 = hs2_all[:, b * F_HID:(b + 1) * F_HID]
                    if b1_nonzero:
                        h = ppool.tile([BLK, F_HID], f32, tag="post1",
                                       name="h")
                        nc.vector.tensor_scalar(out=h[:], in0=x[:],
                                                scalar1=dinv[:, b:b + 1],
                                                scalar2=None, op0=AT.mult)
                        nc.vector.tensor_add(out=h[:], in0=h[:], in1=b1t[:])
                        nc.vector.tensor_scalar(out=sl2, in0=h[:],
                                                scalar1=dinv[:, b:b + 1],
                                                scalar2=0.0, op0=AT.mult,
                                                op1=AT.max)
                    else:
                        nc.scalar.activation(
                            x[:], x[:],
                            func=mybir.ActivationFunctionType.Relu)
                        nc.vector.tensor_scalar(out=sl2, in0=x[:],
                                                scalar1=dinv2[:, b:b + 1],
                                                scalar2=None, op0=AT.mult)
                    hbf = ppool.tile([BLK, TBLW], bf16, tag="post1b",
                                     name="hbf")
                    nc.scalar.activation(
                        hbf[:, :F_HID], sl2,
                        func=mybir.ActivationFunctionType.Copy)
                    pp = piece_of_block(b)
                    r0 = b * BLK - PSTART[pp]
                    nc.sync.dma_start(ag_in[1][pp][r0:r0 + nb, :],
                                      hbf[:nb, :])

            def ag2_piece(p):
                cc = nc.gpsimd.collective_compute(
                    "AllGather", AT.bypass,
                    replica_groups=[list(range(NCORES))],
                    ins=[ag_in[1][p].opt()],
                    outs=[ag_out[1][p].opt()],
                )
                ag2_insts[p] = cc

            run_layer(0, post1, after_pass=ag2_piece)

            # ---- layer 2 post: out = dinv*((bank + dinv*hs2) @ W2) + b2 --
            def post2(bank, bt, bank_blocks):
                for i, b in enumerate(bank_blocks):
                    nb = BLK if b < NBLK - 1 else LAST_BLK
                    sl = bt[:, i * F_HID:(i + 1) * F_HID]
                    hb = hs2_all[:, b * F_HID:(b + 1) * F_HID]
                    ag = ppool.tile([BLK, F_HID], f32, tag="agg2", name="ag")
                    nc.vector.tensor_add(out=ag[:], in0=hb, in1=sl)
                    t2 = pst.tile([BLK, 512], f32, space="PSUM", tag="tmp",
                                  name="t2")
                    nc.tensor.transpose(
                        out=t2[0:F_HID, 0:BLK],
                        in_=ag[:],
                        identity=ident[:])
                    aT = ppool.tile([F_HID, BLK], f32, tag="aggT", name="aT")
                    nc.scalar.activation(aT[:], t2[0:F_HID, 0:BLK],
                                         func=mybir.ActivationFunctionType.Copy)
                    nc.tensor.matmul(out=t2[:, BLK:BLK + F_OUT], lhsT=aT[:],
                                     rhs=W2t[:], start=True, stop=True)
                    o = ppool.tile([BLK, F_OUT], f32, tag="out2", name="o")
                    nc.vector.tensor_scalar(out=o[:],
                                            in0=t2[:, BLK:BLK + F_OUT],
                                            scalar1=dinv[:, b:b + 1],
                                            scalar2=None, op0=AT.mult)
                    if b2_nonzero:
                        nc.vector.tensor_add(out=o[:], in0=o[:], in1=b2t[:])
                    nc.sync.dma_start(y[b * BLK:b * BLK + nb, :], o[:nb, :])

            run_layer(1, post2)

            # Pin collective triggers into chosen spots of the Pool
            # stream (collectives must live on Pool; engine streams are
            # in-order). AG1_1..3 fire right after the first chunks of
            # layer-1 so their meshes run back-to-back on CC while
            # (pass0,w0) gathers execute. AG2_p gets ~3 chunks of
            # desc-gen runway into the next pass so the Pool never
            # stalls waiting for pass-p post writes.
            pins = [((0, 0, 0, 0), ag1_insts.get(1), "AG1_1"),
                    ((0, 0, 1, 0), ag1_insts.get(2), "AG1_2"),
                    ((0, 0, 2, 0), ag1_insts.get(3), "AG1_3"),
                    ((0, 2, 1, 0), ag2_insts.get(0), "AG2_0"),
                    ((0, 4, 0, 1), ag2_insts.get(1), "AG2_1"),
                    ((0, 5, 3, 0), ag2_insts.get(2), "AG2_2"),
                    ((1, 0, 1, 0), ag2_insts.get(3), "AG2_3")]
            for a, cc, nm in pins:
                frm = anchor.get(a)
                if frm is not None and cc is not None:
                    add_dep_helper(frm.ins, cc.ins, True,
                                   f"pin {nm} trigger")

    nc.compile()
    return nc


def _to_bf16(a):
    import ml_dtypes
    return np.asarray(a, dtype=np.float32).astype(ml_dtypes.bfloat16)


def kernel(x, W1, b1, W2, b2, edge_index):
    global LAST_RESULTS
    from concourse.bass_utils import run_bass_kernel_spmd

    x = np.asarray(x, dtype=np.float32)
    W1 = np.asarray(W1, dtype=np.float32)
    W2 = np.asarray(W2, dtype=np.float32)
    b1 = np.asarray(b1, dtype=np.float32)
    b2 = np.asarray(b2, dtype=np.float32)

    ekey = hash(np.asarray(edge_index).tobytes()) ^ hash(
        (bool(np.any(b1)), bool(np.any(b2))))
    if ekey in _CACHE:
        nc, sections, S, per_core = _CACHE[ekey]
    else:
        sections, S, per_core = _preprocess(edge_index)
        nc = _build(sections, S, bool(np.any(b1)), bool(np.any(b2)))
        _CACHE.clear()
        _CACHE[ekey] = (nc, sections, S, per_core)

    b1r = np.broadcast_to(b1, (BLK, F_HID)).copy()
    b2r = np.broadcast_to(b2, (BLK, F_OUT)).copy()
    W1b = _to_bf16(W1)
    iotab = np.tile(np.arange(BLK, dtype=np.int16), (BLK, 1))
    in_maps = []
    for c in range(NCORES):
        pc = per_core[c]
        in_maps.append({
            "xT": _to_bf16(np.ascontiguousarray(x[c * NPC:(c + 1) * NPC].T)),
            "W1": W1b, "W2": W2, "b1r": b1r, "b2r": b2r,
            "degT": pc["degT"], "gidx": pc["gidx"],
            "cs1G": pc["cs1G"], "cs2G": pc["cs2G"], "iotab": iotab,
        })

    res = run_bass_kernel_spmd(nc, in_maps, core_ids=list(range(NCORES)))
    LAST_RESULTS = res
    return np.concatenate([res.results[c]["y"] for c in range(NCORES)], axis=0)


# revision 20
# speedup vs baseline: 1.1529x; 1.1529x over previous
"""Two-layer GCN (BotGCN) on 8 Trainium2 NeuronCores.

Distribution: nodes partitioned contiguously across the 8 cores (12500
each). Each core owns the edges whose destination lands in its block.
Layer math is refactored so all per-edge work is a gather of pre-scaled
rows + a segment-sum:

    out[v] = dinv[v] * (sum_{e: dst=v, real} (dinv[src] * h[src])
                        + dinv[v] * h[v]) + bias

Self-loops are the elementwise term dinv[v]^2 * h[v], added from an
SBUF-resident copy of the local pre-scaled rows.

Structure:
  - Gather table is PIECE-major: local nodes split into 4 pieces
    (3200/3200/3200/2900 + 2 zero rows per core); piece p of the table
    is [8 x PSZE[p] x 256B], produced by its own AllGather so the
    collectives stream piece-by-piece and overlap with compute.
  - Destinations processed in 4 passes of 25/25/25/23 blocks aligned
    with the pieces; layer-1 post for pass p feeds AllGather piece p of
    layer 2 while later passes still compute. Explicit dependency edges
    pin each AG trigger early in the Pool stream (the collective must
    live on the Pool engine) so its ~40us mesh hides under compute.
  - Slots are SEGMENT-packed: per (block, piece) segment rounded to 16
    slots (max over cores), segments concatenated per (pass, piece)
    section and the section padded to 128. 128-slot groups may span two
    blocks; each (group, block) pair gets its own matmul whose one-hot
    is generated on-chip: is_equal(code[slot], bip*128 + c) with
    code = block-in-pass*128 + dst%128 (f32 exact), against an f32
    iota-base table. ~11% padding vs ~21% for per-(block,piece)
    rounding to 128.
  - One DVE is_equal per matmul, 4 dma_gathers per chunk (one per SWDGE
    queue), PE matmuls accumulate into per-pass PSUM banks.
"""

import numpy as np

N = 100000
NCORES = 8
NPC = N // NCORES            # 12500 nodes per core
BLK = 128
NBLK = (NPC + BLK - 1) // BLK          # 98 destination blocks
LAST_BLK = NPC - (NBLK - 1) * BLK      # 84 nodes in the last block
F_IN, F_HID, F_OUT = 128, 64, 2
TBLW = 128                              # bf16 table row width (256B)
NP_ = 4                                 # pieces (gather windows)
NPASS = 6                               # destination passes
PASS_BLOCKS = [list(range(0, 17)), list(range(17, 34)),
               list(range(34, 50)), list(range(50, 66)),
               list(range(66, 82)), list(range(82, NBLK))]
# AG2 piece pp (blocks [25pp,25pp+25)) completes after these passes
AG2_AFTER_PASS = {0: 0, 2: 1, 4: 2, 5: 3}  # pass -> piece
PBLK = [12, 31, 31, 24]                 # blocks per piece (piece 0 tiny so
                                        # the first AllGather fires early)
PBND = [0, 12, 43, 74, 98]              # piece block boundaries
PSTART = [0, 1536, 5504, 9472]          # local node offset of each piece
PSZ = [1536, 3968, 3968, 3028]          # local nodes per piece
PSZE = [q + 2 for q in PSZ]             # + 2 zero rows per core shard
CHUNK_GROUPS = 24                       # max 128-slot groups per chunk

_CACHE = {}

# Results of the most recent run (for the local test harness's profiling).
LAST_RESULTS = None


def _preprocess(edge_index):
    """Host-side integer bucketing of the edge list (self-loops excluded).

    Per (block, piece) segment sized to the max over cores, rounded to
    16 slots; segments concatenated per (pass, piece) section, section
    padded to a 128 multiple. Returns the section descriptors, slot
    count S, and per-core staged gather-index / code arrays.
    """
    src = np.asarray(edge_index[0]).astype(np.int64)
    dst = np.asarray(edge_index[1]).astype(np.int64)

    # degree includes the self-loop (reference semantics)
    deg = (np.bincount(dst, minlength=N) + 1).astype(np.float32)

    core = dst // NPC
    dloc = dst % NPC
    blk = dloc // BLK
    scor = src // NPC
    sloc = src % NPC
    piece = np.searchsorted(np.array(PSTART[1:] + [NPC]), sloc,
                            side="right")
    piece = np.minimum(piece, NP_ - 1)

    cnt = np.zeros((NCORES, NBLK, NP_), np.int64)
    np.add.at(cnt, (core, blk, piece), 1)
    seg = (-(-cnt.max(axis=0) // 16) * 16).astype(np.int64)   # [98, 4]

    seg_id_of = np.zeros((NBLK, NP_), np.int64)
    seg_sizes = []
    seg_win = []
    sections = []
    sid = 0
    slot_base = 0
    for p in range(NPASS):
        for w in range(NP_):
            sec_start = slot_base
            bounds = []
            off = 0
            for b in PASS_BLOCKS[p]:
                seg_id_of[b, w] = sid
                seg_sizes.append(int(seg[b, w]))
                seg_win.append(w)
                bounds.append((b, off, off + int(seg[b, w])))
                off += int(seg[b, w])
                sid += 1
            tot128 = -(-off // 128) * 128
            if tot128 > off:
                seg_sizes.append(tot128 - off)   # section pad pseudo-segment
                seg_win.append(w)
                sid += 1
            slot_base += tot128
            ngroups = tot128 // 128
            mms = []
            for j in range(ngroups):
                lo, hi = j * 128, (j + 1) * 128
                for (b, s0, s1) in bounds:
                    if s0 < hi and s1 > lo:
                        mms.append((j, b))
            byj = {}
            for (j, b) in mms:
                byj.setdefault(j, []).append(b)
            mms_first, mms_cross = [], []
            for j in sorted(byj):
                bs = byj[j]
                assert len(bs) <= 2, bs
                mms_first.append((j, bs[0]))
                if len(bs) > 1:
                    mms_cross.append((j, bs[1]))
            sections.append({"pass": p, "w": w, "ngroups": ngroups,
                             "mms_first": mms_first,
                             "mms_cross": mms_cross,
                             "start_slot": sec_start})
    # processing order: pair passes (0,1),(2,3),(4,5); within a pair,
    # sweep windows and interleave the two passes so each window gives
    # ~2 sections of work to hide one CC AllGather mesh (~42us).
    by_pw = {(sec["pass"], sec["w"]): sec for sec in sections}
    sections = [by_pw[(pp + d, w)]
                for pp in range(0, NPASS, 2)
                for w in range(NP_)
                for d in (0, 1)]
    S = slot_base
    nseg = sid
    seg_sizes = np.array(seg_sizes, np.int64)
    seg_win = np.array(seg_win, np.int64)
    seg_offs = np.zeros(nseg + 1, np.int64)
    np.cumsum(seg_sizes, out=seg_offs[1:])
    win_of_slot = np.repeat(seg_win, seg_sizes)
    zrow = np.array([PSZ[w] for w in range(NP_)], np.int64)

    # per-group base code (first overlapping block-in-pass * 128)
    gbase = np.zeros(S // BLK, np.float32)

    bip = np.zeros(NBLK, np.int64)
    for p in range(NPASS):
        for i, b in enumerate(PASS_BLOCKS[p]):
            bip[b] = i
    for sec in sections:
        g0 = sec["start_slot"] // BLK
        for (j, b) in sec["mms_first"]:
            gbase[g0 + j] = bip[b] * BLK
    base_of_slot = np.repeat(gbase, BLK)

    per_core = []
    for c in range(NCORES):
        m = core == c
        key = seg_id_of[blk[m], piece[m]]
        order = np.argsort(key, kind="stable")
        ks = key[order]
        bstart = np.searchsorted(ks, np.arange(nseg))
        rank = np.arange(len(ks)) - bstart[ks]
        slot = seg_offs[ks] + rank

        so_cor = scor[m][order]
        so_loc = sloc[m][order]
        so_p = piece[m][order]
        row = so_cor * np.array(PSZE)[so_p] + (so_loc - np.array(PSTART)[so_p])

        gidx = zrow[win_of_slot].astype(np.int16)   # pad -> window zero row
        gidx[slot] = row.astype(np.int16)
        code = np.full(S, -1.0, np.float32)
        code[slot] = (bip[blk[m][order]] * BLK
                      + dloc[m][order] % BLK).astype(np.float32)
        # shifted codes: cs1 selects the group's first block (values in
        # [0,128) there), cs2 the second; everything else falls outside
        # [0,128) so the chunk-wide iota is_equal gives all-zero rows.
        cs1 = (code - base_of_slot).astype(np.int16)
        cs2 = (cs1 - BLK).astype(np.int16)

        gidx16 = gidx.reshape(S // 16, 16).T      # [16, S/16]
        gidx_rep = np.tile(gidx16, (8, 1)).copy() # replicated for Q7 cores
        # per slot, [128, S/128]: partition = slot % 128, col = group
        cs1G = np.ascontiguousarray(cs1.reshape(S // BLK, BLK).T)
        cs2G = np.ascontiguousarray(cs2.reshape(S // BLK, BLK).T)
        assert cs1G.dtype == np.int16 and cs2G.dtype == np.int16

        degc = np.ones(NBLK * BLK, np.float32)
        degc[:NPC] = deg[c * NPC:(c + 1) * NPC]
        degT = degc.reshape(NBLK, BLK).T.copy()   # [128, NBLK]

        per_core.append({"gidx": gidx_rep, "cs1G": cs1G, "cs2G": cs2G,
                         "degT": degT})

    return sections, S, per_core


def _chunks_of(n):
    """Split n groups into near-equal chunks of at most CHUNK_GROUPS."""
    k = -(-n // CHUNK_GROUPS)
    base, rem = divmod(n, k)
    out = []
    s = 0
    for i in range(k):
        sz = base + (1 if i < rem else 0)
        out.append((s, sz))
        s += sz
    return out


def _build(sections, S, b1_nonzero, b2_nonzero):
    import concourse.bacc as bacc
    import concourse.mybir as mybir
    import concourse.tile as tile
    from concourse.masks import make_identity
    from bass_rust import add_dep_helper

    f32 = mybir.dt.float32
    bf16 = mybir.dt.bfloat16
    AT = mybir.AluOpType

    bip = {}
    pos = {}
    for p in range(NPASS):
        for i, b in enumerate(PASS_BLOCKS[p]):
            bip[b] = i
            pos[b] = divmod(i, 8)

    # first/last matmul (global emission index) per (pass, bank);
    # emission order: per chunk, first-block matmuls then crossings.
    first, last = {}, {}
    mmidx = 0
    for sec in sections:
        p = sec["pass"]
        for (cs, cn) in _chunks_of(sec["ngroups"]):
            for mlist in (sec["mms_first"], sec["mms_cross"]):
                for (j, b) in mlist:
                    if cs <= j < cs + cn:
                        bank, _ = pos[b]
                        last[(p, bank)] = mmidx
                        first.setdefault((p, bank), mmidx)
                        mmidx += 1

    nc = bacc.Bacc("TRN2", target_bir_lowering=False, debug=False,
                   enable_asserts=False, num_devices=NCORES,
                   num_swdge_queues=4)
    xT = nc.dram_tensor("xT", [F_IN, NPC], bf16, kind="ExternalInput")
    W1 = nc.dram_tensor("W1", [F_IN, F_HID], bf16, kind="ExternalInput")
    W2 = nc.dram_tensor("W2", [F_HID, F_OUT], f32, kind="ExternalInput")
    b1r = nc.dram_tensor("b1r", [BLK, F_HID], f32, kind="ExternalInput")
    b2r = nc.dram_tensor("b2r", [BLK, F_OUT], f32, kind="ExternalInput")
    degT = nc.dram_tensor("degT", [BLK, NBLK], f32, kind="ExternalInput")
    gidx = nc.dram_tensor("gidx", [BLK, S // 16], mybir.dt.int16,
                          kind="ExternalInput")
    cs1G = nc.dram_tensor("cs1G", [BLK, S // BLK], mybir.dt.int16,
                          kind="ExternalInput")
    cs2G = nc.dram_tensor("cs2G", [BLK, S // BLK], mybir.dt.int16,
                          kind="ExternalInput")
    iotab = nc.dram_tensor("iotab", [BLK, BLK], mybir.dt.int16,
                           kind="ExternalInput")
    y = nc.dram_tensor("y", [NPC, F_OUT], f32, kind="ExternalOutput")

    with tile.TileContext(nc) as tc:
        with tc.tile_pool(name="const", bufs=1) as const, \
             tc.tile_pool(name="xt", bufs=3) as xpool, \
             tc.tile_pool(name="hs", bufs=3) as hpool, \
             tc.tile_pool(name="msgs", bufs=10) as mpool, \
             tc.tile_pool(name="oh", bufs=3) as ohpool, \
             tc.tile_pool(name="post", bufs=3) as ppool, \
             tc.tile_pool(name="psb", bufs=2, space="PSUM") as psb, \
             tc.tile_pool(name="pst", bufs=2, space="PSUM") as pst, \
             tc.tile_pool(name="dram", bufs=1, space="DRAM") as dram:

            ag_in = [[dram.tile([PSZE[p], TBLW], bf16,
                                name=f"agin{L}_{p}", tag=f"agin{L}_{p}")
                      for p in range(NP_)] for L in range(2)]
            ag_out = [[dram.tile([8 * PSZE[p], TBLW], bf16,
                                 addr_space="Shared",
                                 name=f"agout{L}_{p}", tag=f"agout{L}_{p}")
                       for p in range(NP_)] for L in range(2)]

            # ---- constants ----
            ident = const.tile([BLK, BLK], f32)
            make_identity(nc, ident[:])
            W1t = const.tile([F_IN, F_HID], bf16)
            nc.sync.dma_start(W1t[:], W1[:])
            W2t = const.tile([F_HID, F_OUT], f32)
            nc.sync.dma_start(W2t[:], W2[:])
            if b1_nonzero:
                b1t = const.tile([BLK, F_HID], f32)
                nc.sync.dma_start(b1t[:], b1r[:])
            if b2_nonzero:
                b2t = const.tile([BLK, F_OUT], f32)
                nc.sync.dma_start(b2t[:], b2r[:])
            degt = const.tile([BLK, NBLK], f32)
            nc.sync.dma_start(degt[:], degT[:])
            rcp = const.tile([BLK, NBLK], f32)
            nc.vector.reciprocal(rcp[:], degt[:])
            dinv = const.tile([BLK, NBLK], f32)
            nc.scalar.sqrt(dinv[:], rcp[:])
            dinv2 = const.tile([BLK, NBLK], f32)
            nc.vector.tensor_mul(dinv2[:], dinv[:], dinv[:])
            idx_sb = const.tile([BLK, S // 16], mybir.dt.int16)
            nc.sync.dma_start(idx_sb[:], gidx[:])
            cs1_sb = const.tile([BLK, S // BLK], mybir.dt.int16)
            nc.sync.dma_start(cs1_sb[:], cs1G[:])
            cs2_sb = const.tile([BLK, S // BLK], mybir.dt.int16)
            nc.sync.dma_start(cs2_sb[:], cs2G[:])
            iota_sb = const.tile([BLK, BLK], mybir.dt.int16)
            nc.sync.dma_start(iota_sb[:], iotab[:])
            zt = const.tile([2, TBLW], bf16)
            nc.gpsimd.memset(zt[:], 0.0)
            for L in range(2):
                for p in range(NP_):
                    nc.sync.dma_start(
                        ag_in[L][p][PSZ[p]:PSZ[p] + 2, :], zt[:])

            # SBUF-resident fp32 copies of the local pre-scaled rows for
            # the elementwise self-loop term (dinv^2 * h == dinv * hs).
            hs1_all = const.tile([BLK, NBLK * F_HID], f32)
            hs2_all = const.tile([BLK, NBLK * F_HID], f32)
            nc.gpsimd.memset(hs1_all[:], 0.0)
            nc.gpsimd.memset(hs2_all[:], 0.0)

            def piece_of_block(b):
                for pp in range(NP_):
                    if b < PBND[pp + 1]:
                        return pp
                return NP_ - 1

            ag1_insts = {}
            ag2_insts = {}

            # ---- phase 1: h_scaled = dinv * (x @ W1), locally owned ----
            for p in range(NP_):
                for t in range(PBND[p], PBND[p + 1]):
                    nt = BLK if t < NBLK - 1 else LAST_BLK
                    xt = xpool.tile([F_IN, BLK], bf16, tag="xt")
                    nc.sync.dma_start(xt[:, :nt], xT[:, t * BLK:t * BLK + nt])
                    hp = pst.tile([BLK, 512], f32, space="PSUM", tag="tmp",
                                  name="hp")
                    nc.tensor.matmul(out=hp[:nt, :F_HID], lhsT=xt[:, :nt],
                                     rhs=W1t[:], start=True, stop=True)
                    nc.vector.tensor_scalar(
                        out=hs1_all[:nt, t * F_HID:(t + 1) * F_HID],
                        in0=hp[:nt, :F_HID],
                        scalar1=dinv[:nt, t:t + 1], scalar2=None,
                        op0=AT.mult)
                    hsb = hpool.tile([BLK, TBLW], bf16, tag="hs")
                    nc.scalar.activation(
                        hsb[:nt, :F_HID],
                        hs1_all[:nt, t * F_HID:(t + 1) * F_HID],
                        func=mybir.ActivationFunctionType.Copy)
                    r0 = t * BLK - PSTART[p]
                    nc.sync.dma_start(ag_in[0][p][r0:r0 + nt, :], hsb[:nt, :])
                cc1 = nc.gpsimd.collective_compute(
                    "AllGather", AT.bypass,
                    replica_groups=[list(range(NCORES))],
                    ins=[ag_in[0][p].opt()],
                    outs=[ag_out[0][p].opt()],
                )
                ag1_insts[p] = cc1

            anchor = {}       # (L, pass, w, chunk) -> first gather inst

            def run_layer(L, post_fn, after_pass=None):
                tables = ag_out[L]
                mmcount = [0]
                banks = {}

                for sec in sections:
                    p, w = sec["pass"], sec["w"]
                    blocks = PASS_BLOCKS[p]
                    if w == 0:
                        banks.clear()
                        for b in blocks:
                            bank, _ = pos[b]
                            if bank not in banks:
                                banks[bank] = psb.tile(
                                    [BLK, 512], f32, space="PSUM",
                                    name=f"bank{bank}", tag=f"bank{bank}")
                    g0_global = sec["start_slot"] // BLK
                    chunks = _chunks_of(sec["ngroups"])
                    for ci, (cs, cn) in enumerate(chunks):
                        mt = mpool.tile([BLK, CHUNK_GROUPS, TBLW], bf16,
                                        tag="msgs")
                        nsub = min(4, cn)
                        base, rem = divmod(cn, nsub)
                        j0 = 0
                        for si in range(nsub):
                            sg = base + (1 if si < rem else 0)
                            if sg == 0:
                                continue
                            sn = sg * BLK
                            soff = g0_global + cs + j0
                            gi = nc.gpsimd.dma_gather(
                                out_ap=mt[:, j0:j0 + sg, :],
                                in_ap=tables[w][:, :],
                                idxs_ap=idx_sb[:, soff * 8:
                                               soff * 8 + sn // 16],
                                num_idxs=sn, num_idxs_reg=sn,
                                elem_size=TBLW,
                                single_packet=False,
                                queue_num=si,
                            )
                            if si == 0:
                                anchor[(L, p, w, ci)] = gi
                            j0 += sg
                        firsts = [(j, b) for (j, b) in sec["mms_first"]
                                  if cs <= j < cs + cn]
                        crosses = [(j, b) for (j, b) in sec["mms_cross"]
                                   if cs <= j < cs + cn]
                        ib = iota_sb[:, :].unsqueeze(1)
                        oh1 = ohpool.tile([BLK, CHUNK_GROUPS, BLK], bf16,
                                          tag="oh1")
                        nc.vector.tensor_tensor(
                            out=oh1[:, :cn, :],
                            in0=ib.to_broadcast([BLK, cn, BLK]),
                            in1=cs1_sb[:, g0_global + cs:
                                       g0_global + cs + cn]
                                .to_broadcast([BLK, cn, BLK]),
                            op=AT.is_equal)
                        if crosses:
                            oh2 = ohpool.tile([BLK, CHUNK_GROUPS, BLK],
                                              bf16, tag="oh2")
                            nc.vector.tensor_tensor(
                                out=oh2[:, :cn, :],
                                in0=ib.to_broadcast([BLK, cn, BLK]),
                                in1=cs2_sb[:, g0_global + cs:
                                           g0_global + cs + cn]
                                    .to_broadcast([BLK, cn, BLK]),
                                op=AT.is_equal)
                        pairs = [(oh1, firsts)]
                        if crosses:
                            pairs.append((oh2, crosses))
                        for oh, mlist in pairs:
                            for (j, b) in mlist:
                                bank, off = pos[b]
                                gm = mmcount[0]
                                mmcount[0] += 1
                                nc.tensor.matmul(
                                    out=banks[bank][:, off * F_HID:
                                                    (off + 1) * F_HID],
                                    lhsT=oh[:, j - cs, :],
                                    rhs=mt[:, j - cs, :F_HID],
                                    start=(gm == first[(p, bank)]),
                                    stop=(gm == last[(p, bank)]),
                                    skip_group_check=True)
                    if w == NP_ - 1:
                        for bank, bt in banks.items():
                            bank_blocks = [b for b in blocks
                                           if pos[b][0] == bank]
                            post_fn(bank, bt, bank_blocks)
                        del pass_banks[p]
                        if after_pass is not None and p in AG2_AFTER_PASS:
                            after_pass(AG2_AFTER_PASS[p])

            # ---- layer 1 post:
            # X = bank + dinv*hs1 (self-loop); h1s = dinv*relu(dinv*X + b1)
            # b1 == 0 fast path: dinv*relu(dinv*X) == dinv^2*relu(X).
            def post1(bank, bt, bank_blocks):
                for i, b in enumerate(bank_blocks):
                    nb = BLK if b < NBLK - 1 else LAST_BLK
                    sl = bt[:, i * F_HID:(i + 1) * F_HID]
                    hb = hs1_all[:, b * F_HID:(b + 1) * F_HID]
                    x = ppool.tile([BLK, F_HID], f32, tag="post1x", name="x")
                    nc.vector.tensor_add(out=x[:], in0=hb, in1=sl)
                    sl2 = hs2_all[:, b * F_HID:(b + 1) * F_HID]
                    if b1_nonzero:
                        h = ppool.tile([BLK, F_HID], f32, tag="post1",
                                       name="h")
                        nc.vector.tensor_scalar(out=h[:], in0=x[:],
                                                scalar1=dinv[:, b:b + 1],
                                                scalar2=None, op0=AT.mult)
                        nc.vector.tensor_add(out=h[:], in0=h[:], in1=b1t[:])
                        nc.vector.tensor_scalar(out=sl2, in0=h[:],
                                                scalar1=dinv[:, b:b + 1],
                                                scalar2=0.0, op0=AT.mult,
                                                op1=AT.max)
                    else:
                        nc.scalar.activation(
                            x[:], x[:],
                            func=mybir.ActivationFunctionType.Relu)
                        nc.vector.tensor_scalar(out=sl2, in0=x[:],
                                                scalar1=dinv2[:, b:b + 1],
                                                scalar2=None, op0=AT.mult)
                    hbf = ppool.tile([BLK, TBLW], bf16, tag="post1b",
                                     name="hbf")
                    nc.scalar.activation(
                        hbf[:, :F_HID], sl2,
                        func=mybir.ActivationFunctionType.Copy)
                    pp = piece_of_block(b)
                    r0 = b * BLK - PSTART[pp]
                    nc.sync.dma_start(ag_in[1][pp][r0:r0 + nb, :],
                                      hbf[:nb, :])

            def ag2_piece(p):
                cc = nc.gpsimd.collective_compute(
                    "AllGather", AT.bypass,
                    replica_groups=[list(range(NCORES))],
                    ins=[ag_in[1][p].opt()],
                    outs=[ag_out[1][p].opt()],
                )
                ag2_insts[p] = cc

            run_layer(0, post1, after_pass=ag2_piece)

            # ---- layer 2 post: out = dinv*((bank + dinv*hs2) @ W2) + b2 --
            def post2(bank, bt, bank_blocks):
                for i, b in enumerate(bank_blocks):
                    nb = BLK if b < NBLK - 1 else LAST_BLK
                    sl = bt[:, i * F_HID:(i + 1) * F_HID]
                    hb = hs2_all[:, b * F_HID:(b + 1) * F_HID]
                    ag = ppool.tile([BLK, F_HID], f32, tag="agg2", name="ag")
                    nc.vector.tensor_add(out=ag[:], in0=hb, in1=sl)
                    t2 = pst.tile([BLK, 512], f32, space="PSUM", tag="tmp",
                                  name="t2")
                    nc.tensor.transpose(
                        out=t2[0:F_HID, 0:BLK],
                        in_=ag[:],
                        identity=ident[:])
                    aT = ppool.tile([F_HID, BLK], f32, tag="aggT", name="aT")
                    nc.scalar.activation(aT[:], t2[0:F_HID, 0:BLK],
                                         func=mybir.ActivationFunctionType.Copy)
                    nc.tensor.matmul(out=t2[:, BLK:BLK + F_OUT], lhsT=aT[:],
                                     rhs=W2t[:], start=True, stop=True)
                    o = ppool.tile([BLK, F_OUT], f32, tag="out2", name="o")
                    nc.vector.tensor_scalar(out=o[:],
                                            in0=t2[:, BLK:BLK + F_OUT],
                                            scalar1=dinv[:, b:b + 1],
                                            scalar2=None, op0=AT.mult)
                    if b2_nonzero:
                        nc.vector.tensor_add(out=o[:], in0=o[:], in1=b2t[:])
                    nc.sync.dma_start(y[b * BLK:b * BLK + nb, :], o[:nb, :])

            run_layer(1, post2)

            # Pin collective triggers into chosen spots of the Pool
            # stream (collectives must live on Pool; engine streams are
            # in-order). AG1_1..3 fire right after the first chunks of
            # layer-1 so their meshes run back-to-back on CC while
            # (pass0,w0) gathers execute. AG2_p gets ~3 chunks of
            # desc-gen runway into the next pass so the Pool never
            # stalls waiting for pass-p post writes.
            pins = [((0, 0, 0, 0), ag1_insts.get(1), "AG1_1"),
                    ((0, 0, 1, 0), ag1_insts.get(2), "AG1_2"),
                    ((0, 0, 2, 0), ag1_insts.get(3), "AG1_3"),
                    ((0, 2, 1, 0), ag2_insts.get(0), "AG2_0"),
                    ((0, 4, 0, 1), ag2_insts.get(1), "AG2_1"),
                    ((1, 0, 0, 1), ag2_insts.get(2), "AG2_2"),
                    ((1, 0, 1, 0), ag2_insts.get(3), "AG2_3")]
            for a, cc, nm in pins:
                frm = anchor.get(a)
                if frm is not None and cc is not None:
                    add_dep_helper(frm.ins, cc.ins, True,
                                   f"pin {nm} trigger")

    nc.compile()
    return nc


def _to_bf16(a):
    import ml_dtypes
    return np.asarray(a, dtype=np.float32).astype(ml_dtypes.bfloat16)


def kernel(x, W1, b1, W2, b2, edge_index):
    global LAST_RESULTS
    from concourse.bass_utils import run_bass_kernel_spmd

    x = np.asarray(x, dtype=np.float32)
    W1 = np.asarray(W1, dtype=np.float32)
    W2 = np.asarray(W2, dtype=np.float32)
    b1 = np.asarray(b1, dtype=np.float32)
    b2 = np.asarray(b2, dtype=np.float32)

    ekey = hash(np.asarray(edge_index).tobytes()) ^ hash(
        (bool(np.any(b1)), bool(np.any(b2))))
    if ekey in _CACHE:
        nc, sections, S, per_core = _CACHE[ekey]
    else:
        sections, S, per_core = _preprocess(edge_index)
        nc = _build(sections, S, bool(np.any(b1)), bool(np.any(b2)))
        _CACHE.clear()
        _CACHE[ekey] = (nc, sections, S, per_core)

    b1r = np.broadcast_to(b1, (BLK, F_HID)).copy()
    b2r = np.broadcast_to(b2, (BLK, F_OUT)).copy()
    W1b = _to_bf16(W1)
    iotab = np.tile(np.arange(BLK, dtype=np.int16), (BLK, 1))
    in_maps = []
    for c in range(NCORES):
        pc = per_core[c]
        in_maps.append({
            "xT": _to_bf16(np.ascontiguousarray(x[c * NPC:(c + 1) * NPC].T)),
            "W1": W1b, "W2": W2, "b1r": b1r, "b2r": b2r,
            "degT": pc["degT"], "gidx": pc["gidx"],
            "cs1G": pc["cs1G"], "cs2G": pc["cs2G"], "iotab": iotab,
        })

    res = run_bass_kernel_spmd(nc, in_maps, core_ids=list(range(NCORES)))
    LAST_RESULTS = res
    return np.concatenate([res.results[c]["y"] for c in range(NCORES)], axis=0)
